# revision 1
# baseline (speedup 1.0000x reference)
"""Deformable-encoder block, data-parallel across 8 NeuronCores.

Sharding: (batch b in 0..3) x (query half) -> 8 shards. Each shard gets its
3200 queries of batch b plus the FULL value[b] grid (the deformable gather can
sample anywhere in the 80x80 grid). Weights are replicated. Outputs are
concatenated back on host. All FLOPs run on device via one pmap'd program.
"""

import functools

import jax
import jax.numpy as jnp
import numpy as np

HEADS = 8
POINTS = 4
EPS = 1e-5

# Problem shape (hardcoded per contract).
N, LQ, C, H, W = 4, 6400, 256, 80, 80
NSH = 8                       # shards / cores
QSH = LQ // 2                 # queries per shard


def _layernorm(x, w, b):
    m = jnp.mean(x, axis=-1, keepdims=True)
    v = jnp.var(x, axis=-1, keepdims=True)
    return (x - m) / jnp.sqrt(v + EPS) * w + b


def _shard_fn(x, ref, value, ln1_w, ln1_b, ln2_w, ln2_b, Wv, bv, Woff, boff,
              Wa, ba, Wout, bout, W1, b1, W2, b2):
    # x: (1, QSH, C); value: (1, H*W, C); ref: (1, QSH, 1, 2)
    xb = x
    a_in = _layernorm(xb, ln1_w, ln1_b)
    n, Lq, c = a_in.shape
    hd = c // HEADS

    v = (value @ Wv + bv).reshape(n, H * W, HEADS, hd).transpose(0, 2, 1, 3)
    off = (a_in @ Woff + boff).reshape(n, Lq, HEADS, POINTS, 2)
    attw = jax.nn.softmax((a_in @ Wa + ba).reshape(n, Lq, HEADS, POINTS), axis=-1)
    offset_normalizer = jnp.array([W, H], dtype=x.dtype)
    loc = ref[:, :, None, :, :] + off / offset_normalizer
    gx = loc[..., 0] * W - 0.5
    gy = loc[..., 1] * H - 0.5
    x0 = jnp.floor(gx)
    y0 = jnp.floor(gy)
    wx = gx - x0
    wy = gy - y0
    x0i = x0.astype(jnp.int32)
    y0i = y0.astype(jnp.int32)

    def gather(xi, yi):
        valid = ((xi >= 0) & (xi < W) & (yi >= 0) & (yi < H))
        idx = (jnp.clip(yi, 0, H - 1) * W + jnp.clip(xi, 0, W - 1)).transpose(0, 2, 1, 3)
        g = jnp.take_along_axis(v, idx.reshape(n, HEADS, Lq * POINTS)[..., None], axis=2)
        return g.reshape(n, HEADS, Lq, POINTS, hd) * valid.transpose(0, 2, 1, 3)[..., None].astype(v.dtype)

    def cw(w_):
        return w_.transpose(0, 2, 1, 3)[..., None]

    samp = (gather(x0i, y0i) * cw((1 - wx) * (1 - wy))
            + gather(x0i + 1, y0i) * cw(wx * (1 - wy))
            + gather(x0i, y0i + 1) * cw((1 - wx) * wy)
            + gather(x0i + 1, y0i + 1) * cw(wx * wy))
    out = jnp.einsum('nhlpd,nlhp->nlhd', samp, attw).reshape(n, Lq, c)
    a = out @ Wout + bout

    xr = xb + a
    h = _layernorm(xr, ln2_w, ln2_b)
    h = jax.nn.gelu(h @ W1 + b1, approximate=False)
    return xr + (h @ W2 + b2)


@functools.partial(jax.pmap, axis_name='i',
                   in_axes=(0, 0, 0) + (None,) * 16)
def _pmapped(x, ref, value, *weights):
    return _shard_fn(x, ref, value, *weights)


def kernel(**inputs) -> np.ndarray:
    x = np.asarray(inputs['x'], np.float32)
    ref = np.asarray(inputs['ref'], np.float32)
    value = np.asarray(inputs['value'], np.float32)
    wnames = ['ln1_w', 'ln1_b', 'ln2_w', 'ln2_b', 'Wv', 'bv', 'Woff', 'boff',
              'Wa', 'ba', 'Wout', 'bout', 'W1', 'b1', 'W2', 'b2']
    weights = [np.asarray(inputs[k], np.float32) for k in wnames]

    # Build shard stacks: shard s = (batch s//2, query half s%2).
    xs = np.stack([x[s // 2, (s % 2) * QSH:(s % 2 + 1) * QSH][None] for s in range(NSH)])
    refs = np.stack([ref[s // 2, (s % 2) * QSH:(s % 2 + 1) * QSH][None] for s in range(NSH)])
    vals = np.stack([value[s // 2][None] for s in range(NSH)])

    out = np.asarray(_pmapped(xs, refs, vals, *weights))  # (8, 1, QSH, C)
    res = np.empty((N, LQ, C), np.float32)
    for s in range(NSH):
        res[s // 2, (s % 2) * QSH:(s % 2 + 1) * QSH] = out[s, 0]
    return res
